# revision 1
# baseline (speedup 1.0000x reference)
"""GNN message-passing kernel for 8 Trainium2 NeuronCores.

Strategy (node-sharded, zero collectives):
  - Pad nodes to 50176 = 8 cores x 49 tiles x 128 slots. A host-side
    "snake deal" permutation assigns nodes to (tile, partition) slots so
    that per-tile edge counts are balanced (the MLP is pointwise, so any
    node permutation is legal; it is undone on the host at the end).
  - Edges are duplicated per direction: mi keyed by dst (gather x[src]),
    mo keyed by src (gather x[dst]). Each direction's edges are bucketed
    by owning tile and by gather-index half (dma_gather indices are
    int16, so x is split into two 25088-row tables), padded to H blocks
    of 128 edges per (tile, half) cell.
  - On-core: bulk dma_gather brings x rows for a group of tiles into
    SBUF; for each 128-edge block a one-hot selection matrix
    S[i, j] = e_i * (part(key_i) == j) is built in one DVE tensor_scalar
    op from an iota constant; PE accumulates psum[d, node] += Y^T @ S
    over the tile's blocks (Y = gathered rows, K=edges contraction).
  - The 4-layer MLP runs feature-major: h = tanh(W^T h + b) via PE
    matmuls with per-partition bias+tanh on the scalar engine. Output is
    written feature-major [128, 6272] per core and re-permuted on host.
"""

import os
import sys

sys.path.insert(0, "/opt/trn_rl_repo")

import numpy as np

from concourse import bass, bacc, mybir, tile
from concourse import bass_utils

N = 50000
E = 800000
D = 128
N_CORES = 8
T_CORE = 49                      # tiles per core
T_TOT = N_CORES * T_CORE         # 392 tiles
NPAD = T_TOT * 128               # 50176
HALF = NPAD // 2                 # 25088 (int16 index limit is 32767)
G = 2                            # tiles per gather group

f32 = mybir.dt.float32
i16 = mybir.dt.int16

LAST_RESULTS = None              # BassKernelResults of the last run


def _register_ntff_hook():
    """Make trace=True work under axon by registering the NTFF profile
    hook that the agent image's antenv package lacks."""
    import types, ctypes, contextlib

    if "antenv.axon_hooks" in sys.modules:
        return
    so_path = "/opt/axon/libaxon_pjrt.so"
    if not os.path.exists(so_path):
        return
    try:
        lib = ctypes.CDLL(so_path)
        if not hasattr(lib, "axon_start_nrt_profile"):
            return
        lib.axon_start_nrt_profile.argtypes = [
            ctypes.POINTER(ctypes.c_int64), ctypes.c_size_t]
        lib.axon_start_nrt_profile.restype = ctypes.c_int64
        lib.axon_stop_nrt_profile.argtypes = [ctypes.c_char_p]
        lib.axon_stop_nrt_profile.restype = ctypes.c_int64

        @contextlib.contextmanager
        def _hook(output_dir, device_ids):
            import jax
            jax.devices()
            if device_ids:
                ids = (ctypes.c_int64 * len(device_ids))(*device_ids)
                rc = lib.axon_start_nrt_profile(ids, len(device_ids))
            else:
                rc = lib.axon_start_nrt_profile(None, 0)
            if rc != 0:
                raise RuntimeError(f"axon_start_nrt_profile rc={rc}")
            try:
                yield
            finally:
                n = lib.axon_stop_nrt_profile(str(output_dir).encode())
                print(f"profile: {n} file(s) -> {output_dir}", file=sys.stderr)

        mod = types.ModuleType("antenv.axon_hooks")
        mod.get_axon_ntff_profile_hook = lambda: _hook
        sys.modules["antenv.axon_hooks"] = mod
    except OSError:
        pass


def _snake_slots(src, dst):
    """Assign each padded node to a (global tile, partition) slot,
    balancing the 4 per-node edge-cell counts across tiles."""
    c1 = np.bincount(dst[src < HALF], minlength=NPAD)
    c2 = np.bincount(dst[src >= HALF], minlength=NPAD)
    c3 = np.bincount(src[dst < HALF], minlength=NPAD)
    c4 = np.bincount(src[dst >= HALF], minlength=NPAD)
    tot = c1 + c2 + c3 + c4
    rank = np.argsort(-tot, kind="stable")
    seq = np.arange(NPAD)
    rounds = seq // T_TOT
    k = seq % T_TOT
    tile_seq = np.where(rounds % 2 == 0, k, T_TOT - 1 - k).astype(np.int32)
    gtile = np.empty(NPAD, np.int32)
    gpart = np.empty(NPAD, np.int32)
    gtile[rank] = tile_seq
    gpart[rank] = rounds.astype(np.int32)
    return gtile, gpart


def _build_dir(key, gat, ew, gtile, gpart, H):
    """Bucket one direction's edges into padded per-(tile, half) cells.

    Returns (gidx [T_TOT, 2, H, 128] int16, ce [T_TOT, 128, 4H] f32)
    where ce columns are [c (2H) | e (2H)] with block col j = half*H+jj.
    """
    t = gtile[key]
    half = (gat >= HALF).astype(np.int64)
    cell = t.astype(np.int64) * 2 + half
    order = np.argsort(cell, kind="stable")
    cell_s = cell[order]
    cnt = np.bincount(cell_s, minlength=T_TOT * 2)
    assert cnt.max() <= H * 128, (cnt.max(), H * 128)
    starts = np.zeros(T_TOT * 2, np.int64)
    starts[1:] = np.cumsum(cnt)[:-1]
    pos = np.arange(len(key)) - starts[cell_s]
    slot = cell_s * (H * 128) + pos

    gidx = np.zeros(T_TOT * 2 * H * 128, np.int16)
    gidx[slot] = (gat[order] - half[order] * HALF).astype(np.int16)
    epad = np.zeros(T_TOT * 2 * H * 128, np.float32)
    epad[slot] = ew[order]
    cpad = np.zeros(T_TOT * 2 * H * 128, np.float32)
    cpad[slot] = gpart[key][order].astype(np.float32)

    gidx = gidx.reshape(T_TOT, 2, H, 128)
    # block col j = half*H + jj, partition p = edge index within block
    c_t = cpad.reshape(T_TOT, 2, H, 128).transpose(0, 3, 1, 2).reshape(
        T_TOT, 128, 2 * H)
    e_t = epad.reshape(T_TOT, 2, H, 128).transpose(0, 3, 1, 2).reshape(
        T_TOT, 128, 2 * H)
    ce = np.concatenate([c_t, e_t], axis=2)  # [T_TOT, 128, 4H]
    return gidx, ce


def _wrap_idx(arr):
    """[L] int16 -> [128, L//16] in the dma_gather layout: idx i at
    [i % 16, i // 16], replicated across the 8 Q7 core stripes."""
    L = arr.shape[0]
    w = arr.reshape(L // 16, 16).T  # [16, L//16]
    return np.ascontiguousarray(np.tile(w, (8, 1)))


def _preprocess(x, e, edge_index):
    src = np.asarray(edge_index[0], np.int64)
    dst = np.asarray(edge_index[1], np.int64)
    ew = np.asarray(e, np.float32)
    xpad = np.zeros((NPAD, D), np.float32)
    xpad[:N] = np.asarray(x, np.float32)

    gtile, gpart = _snake_slots(src, dst)

    # one H for the whole (uniform SPMD) program
    def _max_cell(key, gat):
        cell = gtile[key].astype(np.int64) * 2 + (gat >= HALF)
        return np.bincount(cell, minlength=T_TOT * 2).max()

    H = int(np.ceil(max(_max_cell(dst, src), _max_cell(src, dst)) / 128))

    gidx_mi, ce_mi = _build_dir(dst, src, ew, gtile, gpart, H)
    gidx_mo, ce_mo = _build_dir(src, dst, ew, gtile, gpart, H)
    ce = np.ascontiguousarray(
        np.concatenate([ce_mi, ce_mo], axis=2))  # [T_TOT, 128, 8H]

    # feature-major x in slot order for the MLP concat input
    perm_nodes = np.empty(NPAD, np.int64)
    gslot = gtile.astype(np.int64) * 128 + gpart
    perm_nodes[gslot] = np.arange(NPAD)
    xpermT = np.ascontiguousarray(xpad[perm_nodes].T)  # [128, NPAD]

    iota = np.broadcast_to(np.arange(128, dtype=np.float32),
                           (128, 128)).copy()

    per_core = []
    for k in range(N_CORES):
        ts = slice(k * T_CORE, (k + 1) * T_CORE)
        m = {
            "x_lo": xpad[:HALF],
            "x_hi": xpad[HALF:],
            "xT": np.ascontiguousarray(
                xpermT[:, k * T_CORE * 128:(k + 1) * T_CORE * 128]),
            "ce": np.ascontiguousarray(ce[ts]),
            "iota": iota,
        }
        for dname, gi in (("mi", gidx_mi), ("mo", gidx_mo)):
            for h in (0, 1):
                flat = gi[ts, h].reshape(-1)  # [T_CORE*H*128]
                m[f"idx_{dname}{h}"] = _wrap_idx(flat)
        per_core.append(m)
    return per_core, gslot, H


_NC_CACHE = {}


def _build_nc(H):
    if H in _NC_CACHE:
        return _NC_CACHE[H]
    HB = 2 * H  # blocks per (tile, direction)
    nc = bacc.Bacc("TRN2", target_bir_lowering=False, debug=False,
                   enable_asserts=False, num_devices=N_CORES,
                   num_swdge_queues=4)

    x_lo = nc.dram_tensor("x_lo", [HALF, D], f32, kind="ExternalInput").ap()
    x_hi = nc.dram_tensor("x_hi", [HALF, D], f32, kind="ExternalInput").ap()
    xT = nc.dram_tensor("xT", [128, T_CORE * 128], f32,
                        kind="ExternalInput").ap()
    ce = nc.dram_tensor("ce", [T_CORE, 128, 8 * H], f32,
                        kind="ExternalInput").ap()
    iota_d = nc.dram_tensor("iota", [128, 128], f32,
                            kind="ExternalInput").ap()
    idx = {}
    for dname in ("mi", "mo"):
        for h in (0, 1):
            idx[(dname, h)] = nc.dram_tensor(
                f"idx_{dname}{h}", [128, T_CORE * H * 8], i16,
                kind="ExternalInput").ap()
    w1 = nc.dram_tensor("W1", [3 * D, D], f32, kind="ExternalInput").ap()
    wds = {2: nc.dram_tensor("W2", [D, D], f32, kind="ExternalInput").ap(),
           3: nc.dram_tensor("W3", [D, D], f32, kind="ExternalInput").ap(),
           4: nc.dram_tensor("W4", [D, D], f32, kind="ExternalInput").ap()}
    bds = {i: nc.dram_tensor(f"b{i}", [D], f32, kind="ExternalInput").ap()
           for i in (1, 2, 3, 4)}
    out_t = nc.dram_tensor("out_t", [128, T_CORE * 128], f32,
                           kind="ExternalOutput").ap()

    eq = mybir.AluOpType.is_equal
    mul = mybir.AluOpType.mult
    tanh = mybir.ActivationFunctionType.Tanh

    with tile.TileContext(nc) as tc:
        with (
            tc.tile_pool(name="const", bufs=1) as cpool,
            tc.tile_pool(name="gath", bufs=6) as gpool,
            tc.tile_pool(name="idxp", bufs=6) as ipool,
            tc.tile_pool(name="work", bufs=3) as wpool,
            tc.tile_pool(name="sel", bufs=6) as spool,
            tc.tile_pool(name="hbuf", bufs=3) as hpool,
            tc.tile_pool(name="ps", bufs=4, space="PSUM") as pspool,
            tc.tile_pool(name="psm", bufs=2, space="PSUM") as mpool,
        ):
            iota_t = cpool.tile([128, 128], f32)
            nc.sync.dma_start(out=iota_t[:], in_=iota_d[:, :])
            wt = {}
            for j in range(3):
                wt[(1, j)] = cpool.tile([128, 128], f32, tag=f"w1{j}",
                                        name=f"w1{j}")
                nc.sync.dma_start(out=wt[(1, j)][:],
                                  in_=w1[j * 128:(j + 1) * 128, :])
            for i in (2, 3, 4):
                wt[i] = cpool.tile([128, 128], f32, tag=f"w{i}",
                                   name=f"w{i}")
                nc.sync.dma_start(out=wt[i][:], in_=wds[i][:, :])
            bt = {}
            for i in (1, 2, 3, 4):
                bt[i] = cpool.tile([128, 1], f32, tag=f"b{i}",
                                   name=f"b{i}")
                nc.sync.dma_start(out=bt[i][:], in_=bds[i][:, None])

            # Gathers run as rolling 8-block (1024-idx) chunks so each call
            # fits one SWDGE packet per SDMA engine (single_packet=True) and
            # dispatches in ~100-400ns on the engine row; desc-gen runs
            # async on the Q7 core pair (2q, 2q+1) selected by queue_num,
            # round-robin over all 4 queues.
            CHUNK = 8                      # blocks per gather (<= 1024 idxs)
            NBLK = T_CORE * H              # blocks per (dir, half) stream
            streams = [("mi", 0), ("mi", 1), ("mo", 0), ("mo", 1)]
            chunks = {s: [] for s in streams}   # chunk tiles per stream
            next_chunk = {s: 0 for s in streams}
            qrr = [0]

            def emit_chunks(upto_block):
                for s in streams:
                    dname, h = s
                    while (next_chunk[s] * CHUNK < upto_block
                           and next_chunk[s] * CHUNK < NBLK):
                        c = next_chunk[s]
                        nb = min(CHUNK, NBLK - c * CHUNK)
                        nidx = nb * 128
                        it = ipool.tile([128, nb * 8], i16,
                                        tag=f"i{dname}{h}",
                                        name=f"i{dname}{h}")
                        nc.sync.dma_start(
                            out=it[:],
                            in_=idx[s][:, c * CHUNK * 8:
                                       (c * CHUNK + nb) * 8])
                        gb = gpool.tile([128, nb, 128], f32,
                                        tag=f"g{dname}{h}",
                                        name=f"g{dname}{h}")
                        q = (qrr[0] + 1) % 4   # rotate 1,2,3,0,...
                        qrr[0] = q
                        nc.gpsimd.dma_gather(
                            out_ap=gb[:],
                            in_ap=(x_lo if h == 0 else x_hi)[:, :],
                            idxs_ap=it[:],
                            num_idxs=nidx,
                            num_idxs_reg=nidx,
                            elem_size=D,
                            single_packet=True,
                            queue_num=q,
                        )
                        chunks[s].append(gb)
                        next_chunk[s] += 1

            if True:
                for t in range(T_CORE):
                    emit_chunks(min((t + 2) * H, NBLK))
                    cet = wpool.tile([128, 8 * H], f32, tag="ce")
                    nc.sync.dma_start(out=cet[:], in_=ce[t])
                    xt_t = wpool.tile([128, 128], f32, tag="xt")
                    nc.sync.dma_start(
                        out=xt_t[:], in_=xT[:, t * 128:(t + 1) * 128])

                    acc = {}
                    for di, dname in enumerate(("mi", "mo")):
                        ps = pspool.tile([128, 128], f32, tag="scat")
                        cbase = di * 4 * H
                        ebase = cbase + 2 * H
                        for j in range(HB):
                            h = 0 if j < H else 1
                            jj = j - h * H
                            s_t = spool.tile([128, 128], f32, tag="s")
                            nc.vector.scalar_tensor_tensor(
                                s_t[:], iota_t[:],
                                cet[:, cbase + j:cbase + j + 1],
                                cet[:, ebase + j:ebase + j + 1]
                                .to_broadcast([128, 128]),
                                eq, mul)
                            blk = t * H + jj
                            y = chunks[(dname, h)][blk // CHUNK][
                                :, blk % CHUNK, :]
                            nc.tensor.matmul(
                                out=ps[:], lhsT=y, rhs=s_t[:],
                                start=(j == 0), stop=(j == HB - 1))
                        acc[dname] = hpool.tile([128, 128], f32,
                                                tag=f"acc{dname}",
                                                name=f"acc{dname}")
                        nc.scalar.copy(out=acc[dname][:], in_=ps[:])

                    hp = mpool.tile([128, 128], f32, tag="mlp")
                    nc.tensor.matmul(out=hp[:], lhsT=wt[(1, 0)][:],
                                     rhs=acc["mi"][:], start=True, stop=False)
                    nc.tensor.matmul(out=hp[:], lhsT=wt[(1, 1)][:],
                                     rhs=acc["mo"][:], start=False, stop=False)
                    nc.tensor.matmul(out=hp[:], lhsT=wt[(1, 2)][:],
                                     rhs=xt_t[:], start=False, stop=True)
                    hprev = hpool.tile([128, 128], f32, tag="h")
                    nc.scalar.activation(hprev[:], hp[:], tanh,
                                         bias=bt[1][:, 0:1])
                    for i in (2, 3, 4):
                        hp = mpool.tile([128, 128], f32, tag="mlp")
                        nc.tensor.matmul(out=hp[:], lhsT=wt[i][:],
                                         rhs=hprev[:], start=True, stop=True)
                        hnext = hpool.tile([128, 128], f32, tag="h")
                        nc.scalar.activation(hnext[:], hp[:], tanh,
                                             bias=bt[i][:, 0:1])
                        hprev = hnext
                    nc.sync.dma_start(
                        out=out_t[:, t * 128:(t + 1) * 128], in_=hprev[:])

    nc.compile()
    _NC_CACHE[H] = nc
    return nc


def kernel(**inputs):
    global LAST_RESULTS
    _register_ntff_hook()
    x = np.asarray(inputs["x"], np.float32)
    e = np.asarray(inputs["e"], np.float32)
    edge_index = np.asarray(inputs["edge_index"])

    per_core, gslot, H = _preprocess(x, e, edge_index)
    nc = _build_nc(H)

    shared = {"W1": np.asarray(inputs["W1"], np.float32)}
    for i in (2, 3, 4):
        shared[f"W{i}"] = np.asarray(inputs[f"W{i}"], np.float32)
    for i in (1, 2, 3, 4):
        shared[f"b{i}"] = np.asarray(inputs[f"b{i}"], np.float32)

    in_maps = []
    for k in range(N_CORES):
        m = dict(per_core[k])
        m.update(shared)
        in_maps.append(m)

    res = bass_utils.run_bass_kernel_spmd(nc, in_maps,
                                          core_ids=list(range(N_CORES)))
    LAST_RESULTS = res
    big = np.concatenate([res.results[k]["out_t"] for k in range(N_CORES)],
                         axis=1)  # [128, NPAD] feature-major, slot order
    out = big.T[gslot[:N]]
    return np.ascontiguousarray(out.astype(np.float32))

